# revision 9
# baseline (speedup 1.0000x reference)
"""GCN layer (GCNConv + log_softmax) on 8 Trainium2 NeuronCores.

v2 design (vs baseline):
- nodes row-sharded 8 ways; h' = (dis*x) @ W computed in bf16 (dis folded
  into x on host), written chunk-wise to DRAM.
- 4 source chunks, each AllGathered as soon as its GEMM tiles finish, so
  the collective stream overlaps the GEMM and the aggregation of earlier
  chunks.
- aggregation gathers message source rows from the AllGathered chunk with
  prepare_only SWDGE gathers: descriptors are generated on the idle GpSimd
  engine ahead of time across 4 SWDGE queues; trigger_dma fires them the
  moment the chunk's AllGather lands.  Manual completion semaphores gate
  the consumer matmuls (Tile's automatic DMASW tracking is not reliable
  for prepped gathers).
- per (chunk, tile) blocks of 128 messages are combined with host-built
  0/1 one-hot matrices on the tensor engine, accumulating per-tile partial
  sums in SBUF (bf16).
- epilogue: z = partial*dis + bias, exp with accumulate (one table load),
  one Ln over all tiles' sums, then res = partial*dis - lse + bias, written
  as bf16 (host converts to f32).  No max-subtraction: |z| <= ~5.
"""

import numpy as np
import ml_dtypes

import concourse.bass as bass
import concourse.tile as tile
from concourse import bacc, mybir
from concourse.bass_utils import run_bass_kernel_spmd

bf16 = ml_dtypes.bfloat16
F32 = mybir.dt.float32
BF16 = mybir.dt.bfloat16
I16 = mybir.dt.int16

N_NODES = 50000
D_IN = 2048
D_OUT = 512
C = 8                       # cores
NLOC = N_NODES // C         # 6250 real nodes per core
T = 49                      # dst tiles per core
NPAD = T * 128              # 6272 padded rows per core
KT = D_IN // 128            # 16 contraction chunks
CH = 4                      # source chunks (AllGather granularity)
CH_T0 = [0, 12, 24, 36]     # first tile of each chunk
CH_NT = [12, 12, 12, 13]    # tiles per chunk
CH_OFF = [0, 1536, 3072, 4608]
CH_SZ = [1536, 1536, 1536, 1664]
GSZ = [8 * s for s in CH_SZ]   # global chunk rows (< 32768 -> int16 ok)
GB = 8                      # blocks per gather group
NQ = 4                      # SWDGE queues
NSEM = 16                   # rotating gather-completion semaphores
SLAB = 4                    # GEMM tiles per xk slab

LAST_RESULTS = None         # test harness reads exec_time_ns from here


def _wrap_idx(idx):
    """Wrap a [n] index array into the [128, n//16] dma_gather layout."""
    n = idx.shape[0]
    assert n % 16 == 0
    cols = n // 16
    w = np.empty((128, cols), np.int16)
    blk = idx.reshape(cols, 16).T.astype(np.int16)
    for g in range(8):
        w[g * 16:(g + 1) * 16, :] = blk
    return w


def _preprocess(x, edge_index, weight, bias):
    src = np.asarray(edge_index[0], dtype=np.int64)
    dst = np.asarray(edge_index[1], dtype=np.int64)
    loops = np.arange(N_NODES, dtype=np.int64)
    msrc = np.concatenate([src, loops])
    mdst = np.concatenate([dst, loops])

    deg = np.bincount(mdst, minlength=N_NODES).astype(np.float32)
    dis = 1.0 / np.sqrt(deg)          # deg >= 1 because of self loops

    sc = msrc // NLOC
    sr = msrc % NLOC
    k = np.searchsorted(np.array(CH_OFF), sr, side="right") - 1
    off = np.array(CH_OFF)[k]
    g = sc * np.array(CH_SZ)[k] + (sr - off)

    dc = mdst // NLOC
    dr = mdst % NLOC
    t = dr // 128
    dl = dr % 128

    order = np.lexsort((g, t, k, dc))
    g, k, t, dl, dc = g[order], k[order], t[order], dl[order], dc[order]

    key = (dc * CH + k) * T + t
    counts = np.bincount(key, minlength=C * CH * T).reshape(C, CH, T)
    B = (-(-counts // 128)).max(axis=0)          # [CH, T] blocks, SPMD-uniform
    assert (B.sum(axis=0) > 0).all()

    totB = int(B.sum())
    first_k = np.argmax(B > 0, axis=0)           # first chunk with blocks, per tile

    starts = np.zeros(C * CH * T + 1, np.int64)
    np.cumsum(np.bincount(key, minlength=C * CH * T), out=starts[1:])

    # global block base per (k, t), chunk-major then tile-major
    bbase = np.zeros((CH, T), np.int64)
    bb = 0
    for kk in range(CH):
        for tt in range(T):
            bbase[kk, tt] = bb
            bb += int(B[kk, tt])

    idx_cols = 8 * totB
    idx_np = np.zeros((C, 128, idx_cols), np.int16)
    oh_np = np.zeros((C, 128, totB * 128), bf16)

    for c in range(C):
        gidx = np.zeros(totB * 128, np.int64)
        for kk in range(CH):
            for tt in range(T):
                nb = int(B[kk, tt])
                if nb == 0:
                    continue
                s0, s1 = starts[(c * CH + kk) * T + tt], starts[(c * CH + kk) * T + tt + 1]
                cnt = int(s1 - s0)
                pp = np.arange(cnt)
                base = int(bbase[kk, tt])
                gidx[base * 128 + pp] = g[s0:s1]
                oh_np[c, pp % 128, (base + pp // 128) * 128 + dl[s0:s1]] = 1.0
        # wrap indices per gather group (groups are per chunk, GB blocks each)
        for kk in range(CH):
            b0 = int(bbase[kk, 0])
            bn = int(B[kk].sum())
            gs = b0
            while gs < b0 + bn:
                nb = min(GB, b0 + bn - gs)
                idx_np[c, :, 8 * gs:8 * (gs + nb)] = _wrap_idx(
                    gidx[gs * 128:(gs + nb) * 128])
                gs += nb

    # GEMM inputs: x pre-scaled by dis, transposed, bf16
    xT = np.zeros((C, D_IN, NPAD), bf16)
    dis_np = np.zeros((C, 128, T), np.float32)
    for c in range(C):
        xs = x[c * NLOC:(c + 1) * NLOC] * dis[c * NLOC:(c + 1) * NLOC, None]
        xT[c, :, :NLOC] = xs.T.astype(bf16)
        dis_np[c, :, :] = np.pad(dis[c * NLOC:(c + 1) * NLOC],
                                 (0, NPAD - NLOC)).reshape(T, 128).T

    w_bf = np.ascontiguousarray(weight.astype(bf16))
    bias_full = np.tile(np.asarray(bias, np.float32)[None, :], (128, 1))

    return dict(
        B=B, first_k=first_k, bbase=bbase,
        idx=idx_np, oh=oh_np, w=w_bf, xT=xT,
        dis=dis_np, bias=np.ascontiguousarray(bias_full),
    )


def _build(B, first_k, bbase):
    totB = int(B.sum())
    idx_cols = 8 * totB

    nc = bacc.Bacc("TRN2", target_bir_lowering=False, debug=False,
                   num_devices=C, num_swdge_queues=NQ)

    xT_t = nc.dram_tensor("xT", [D_IN, NPAD], BF16, kind="ExternalInput")
    w_t = nc.dram_tensor("w", [D_IN, D_OUT], BF16, kind="ExternalInput")
    dis_t = nc.dram_tensor("dis", [128, T], F32, kind="ExternalInput")
    bias_t = nc.dram_tensor("biasf", [128, D_OUT], F32, kind="ExternalInput")
    idx_t = nc.dram_tensor("idx", [128, idx_cols], I16, kind="ExternalInput")
    oh_t = nc.dram_tensor("oh", [128, totB * 128], BF16, kind="ExternalInput")
    out_t = nc.dram_tensor("out", [NPAD, D_OUT], BF16, kind="ExternalOutput")

    xT, w, dis, biasf, idx, oh, out = (
        t.ap() for t in (xT_t, w_t, dis_t, bias_t, idx_t, oh_t, out_t))

    # group table per chunk: (global block start, nblocks)
    groups = []          # flat list over chunks
    grp_of_block = {}
    chunk_groups = [[] for _ in range(CH)]
    for kk in range(CH):
        b0 = int(bbase[kk, 0])
        bn = int(B[kk].sum())
        gs = b0
        while gs < b0 + bn:
            nb = min(GB, b0 + bn - gs)
            gi = len(groups)
            groups.append((gs, nb))
            chunk_groups[kk].append(gi)
            for b in range(gs, gs + nb):
                grp_of_block[b] = gi
            gs += nb

    with tile.TileContext(nc) as tc:
        with tc.tile_pool(name="const", bufs=1) as constp, \
             tc.tile_pool(name="xk", bufs=2) as xkp, \
             tc.tile_pool(name="hc", bufs=3) as hcp, \
             tc.tile_pool(name="gath", bufs=8) as gp, \
             tc.tile_pool(name="ohp", bufs=6) as ohp, \
             tc.tile_pool(name="zt", bufs=2) as zp, \
             tc.tile_pool(name="res", bufs=2) as resp, \
             tc.tile_pool(name="gps", bufs=3, space="PSUM") as gpsp, \
             tc.tile_pool(name="aps", bufs=3, space="PSUM") as apsp, \
             tc.tile_pool(name="eps", bufs=2, space="PSUM") as epsp, \
             tc.tile_pool(name="dram", bufs=1, space="DRAM") as dramp:

            # ---- resident constants ----
            w_sb = constp.tile([128, KT, D_OUT], BF16)
            for kt in range(KT):
                nc.sync.dma_start(out=w_sb[:, kt, :],
                                  in_=w[kt * 128:(kt + 1) * 128, :])
            dis_sb = constp.tile([128, T], F32)
            nc.sync.dma_start(out=dis_sb[:], in_=dis[:])
            bias_sb = constp.tile([128, D_OUT], F32)
            nc.sync.dma_start(out=bias_sb[:], in_=biasf[:])
            idx_sb = constp.tile([128, idx_cols], I16)
            nc.sync.dma_start(out=idx_sb[:], in_=idx[:])

            partial = constp.tile([128, T, D_OUT], BF16)
            sums = constp.tile([128, T], F32)
            lse = constp.tile([128, T], F32)
            nlse = constp.tile([128, T], F32)

            h_loc = [dramp.tile([CH_SZ[kk], D_OUT], BF16, name=f"h_loc{kk}")
                     for kk in range(CH)]
            h_g = [dramp.tile([GSZ[kk], D_OUT], BF16, addr_space="Shared",
                              name=f"h_g{kk}")
                   for kk in range(CH)]

            # semaphores are locked to one SWDGE queue each: per-queue pools
            SPQ = NSEM // NQ
            sems = [[nc.alloc_semaphore(f"gsem{q}_{i}") for i in range(SPQ)]
                    for q in range(NQ)]
            sem_uses = [[0] * SPQ for _ in range(NQ)]
            sem_rr = [0] * NQ
            grp_wait = {}      # gi -> (sem, value)

            # ---- phase 1: GEMM h' = (dis*x) @ W, chunk-wise + AllGathers ----
            nslab = (T + SLAB - 1) // SLAB
            for s in range(nslab):
                t0 = s * SLAB
                nt = min(SLAB, T - t0)
                xk = xkp.tile([128, KT, SLAB * 128], BF16, name="xk")
                for kt in range(KT):
                    nc.sync.dma_start(
                        out=xk[:, kt, :nt * 128],
                        in_=xT[kt * 128:(kt + 1) * 128,
                               t0 * 128:(t0 + nt) * 128])
                for ti in range(nt):
                    t_ = t0 + ti
                    ph = gpsp.tile([128, D_OUT], F32, name="ph")
                    for kt in range(KT):
                        nc.tensor.matmul(
                            ph[:], xk[:, kt, ti * 128:(ti + 1) * 128],
                            w_sb[:, kt, :], start=(kt == 0), stop=(kt == KT - 1))
                    hcv = hcp.tile([128, D_OUT], BF16, name="hcv")
                    nc.vector.tensor_copy(hcv[:], ph[:])
                    kk = next(i for i in range(CH)
                              if CH_T0[i] <= t_ < CH_T0[i] + CH_NT[i])
                    r0 = (t_ - CH_T0[kk]) * 128
                    r1 = min(r0 + 128, CH_SZ[kk])
                    nc.sync.dma_start(out=h_loc[kk][r0:r1, :],
                                      in_=hcv[:r1 - r0, :])

            # ---- phase 2: gpsimd program: preps, AG triggers, dma triggers --
            rg = [list(range(C))]

            def emit_ag(kk):
                nc.gpsimd.collective_compute(
                    "AllGather", mybir.AluOpType.bypass,
                    replica_groups=rg,
                    ins=[h_loc[kk].opt()], outs=[h_g[kk].opt()])

            gtiles = {}

            def emit_prep_batch(kk, gis):
                for j, gi in enumerate(gis):
                    gs, nb = groups[gi]
                    gt = gp.tile([128, GB, D_OUT], BF16, name="gt", tag="gt")
                    gtiles[gi] = gt
                    q = j % NQ
                    si = sem_rr[q] % SPQ
                    sem_rr[q] += 1
                    sem_uses[q][si] += 1
                    grp_wait[gi] = (sems[q][si], 16 * sem_uses[q][si])
                    nc.gpsimd.dma_gather(
                        out_ap=gt[:, :nb, :], in_ap=h_g[kk][:],
                        idxs_ap=idx_sb[:, 8 * gs:8 * (gs + nb)],
                        num_idxs=nb * 128, num_idxs_reg=nb * 128,
                        elem_size=D_OUT, prepare_only=True, sem=sems[q][si],
                        queue_num=q)

            def emit_trigger_batch(gis):
                for j in range(min(len(gis), NQ)):
                    nc.gpsimd.trigger_dma(count=None, queue_num=j)

            # batches of NQ groups per chunk.  AG_k is emitted BEFORE chunk
            # k's preps (its h_g write must precede their h_g read in
            # program order); AG_{k+1} is emitted right after chunk k's
            # first trigger batch, which waits for AG_k completion -- so
            # the collective stream runs back-to-back.
            batches = [[chunk_groups[kk][i:i + NQ]
                        for i in range(0, len(chunk_groups[kk]), NQ)]
                       for kk in range(CH)]

            emit_ag(0)
            for kk in range(CH):
                for bi, gis in enumerate(batches[kk]):
                    emit_prep_batch(kk, gis)
                    emit_trigger_batch(gis)
                    if bi == 0 and kk + 1 < CH:
                        emit_ag(kk + 1)

            # ---- phase 3: aggregation matmuls + partial accumulation ----
            _oh_tiles = {}
            waited = set()
            for kk in range(CH):
                for tt in range(T):
                    nb_t = int(B[kk, tt])
                    if nb_t == 0:
                        continue
                    ps = apsp.tile([128, D_OUT], F32, name="ps")
                    base = int(bbase[kk, tt])
                    for b in range(nb_t):
                        bi_ = base + b
                        gi = grp_of_block[bi_]
                        if gi not in waited:
                            sem, val = grp_wait[gi]
                            nc.tensor.wait_ge(sem, val)
                            waited.add(gi)
                        gs, nb = groups[gi]
                        slot = bi_ - gs
                        oht = _oh_tiles.get(gi)
                        if oht is None:
                            oht = ohp.tile([128, GB * 128], BF16, name="oht")
                            nc.scalar.dma_start(
                                out=oht[:, :nb * 128],
                                in_=oh[:, gs * 128:(gs + nb) * 128])
                            _oh_tiles[gi] = oht
                        nc.tensor.matmul(
                            ps[:], oht[:, slot * 128:(slot + 1) * 128],
                            gtiles[gi][:, slot, :],
                            start=(b == 0), stop=(b == nb_t - 1))
                    if kk == first_k[tt]:
                        nc.vector.tensor_copy(partial[:, tt, :], ps[:])
                    else:
                        nc.vector.tensor_tensor(
                            partial[:, tt, :], partial[:, tt, :], ps[:],
                            mybir.AluOpType.add)
                    # epilogue sweep 1 for this tile, once its last chunk done
                    if kk == CH - 1 or all(B[k2, tt] == 0
                                           for k2 in range(kk + 1, CH)):
                        zt = zp.tile([128, D_OUT], F32, name="zt")
                        nc.vector.tensor_scalar(
                            zt[:], partial[:, tt, :], dis_sb[:, tt:tt + 1],
                            None, mybir.AluOpType.mult)
                        nc.vector.tensor_tensor(zt[:], zt[:], bias_sb[:],
                                                mybir.AluOpType.add)
                        ep = epsp.tile([128, D_OUT], F32, name="ep")
                        nc.scalar.activation(
                            ep[:], zt[:], mybir.ActivationFunctionType.Exp,
                            accum_out=sums[:, tt:tt + 1])

            # ---- phase 4: lse + final sweep ----
            nc.scalar.activation(lse[:], sums[:],
                                 mybir.ActivationFunctionType.Ln)
            nc.vector.tensor_scalar(nlse[:], lse[:], -1.0, None,
                                    mybir.AluOpType.mult)
            for tt in range(T):
                zt = zp.tile([128, D_OUT], F32, name="zt2")
                nc.vector.tensor_scalar(
                    zt[:], partial[:, tt, :], dis_sb[:, tt:tt + 1],
                    nlse[:, tt:tt + 1], mybir.AluOpType.mult,
                    mybir.AluOpType.add)
                res = resp.tile([128, D_OUT], BF16, name="res")
                nc.vector.tensor_tensor(res[:], zt[:], bias_sb[:],
                                        mybir.AluOpType.add)
                nc.scalar.dma_start(out=out[tt * 128:(tt + 1) * 128, :],
                                    in_=res[:])

    nc.compile()
    return nc


def kernel(x, edge_index, weight, bias):
    global LAST_RESULTS
    x = np.asarray(x, dtype=np.float32)
    weight = np.asarray(weight, dtype=np.float32)
    bias = np.asarray(bias, dtype=np.float32)

    pp = _preprocess(x, edge_index, weight, bias)
    nc = _build(pp["B"], pp["first_k"], pp["bbase"])

    in_maps = []
    for c in range(C):
        in_maps.append({
            "xT": np.ascontiguousarray(pp["xT"][c]),
            "w": pp["w"],
            "dis": np.ascontiguousarray(pp["dis"][c]),
            "biasf": pp["bias"],
            "idx": np.ascontiguousarray(pp["idx"][c]),
            "oh": np.ascontiguousarray(pp["oh"][c]),
        })

    res = run_bass_kernel_spmd(nc, in_maps, core_ids=list(range(C)))
    LAST_RESULTS = res

    out = np.empty((N_NODES, D_OUT), np.float32)
    for c in range(C):
        out[c * NLOC:(c + 1) * NLOC] = \
            res.results[c]["out"][:NLOC].astype(np.float32)
    return out


# revision 11
# speedup vs baseline: 1.1160x; 1.1160x over previous
"""GCN layer (GCNConv + log_softmax) on 8 Trainium2 NeuronCores.

v3 design:
- nodes row-sharded 8 ways; h' = (dis*x) @ W in bf16 (dis folded into x on
  host).  xT is pre-tiled on host into the exact SBUF slab layout so each
  slab loads with one full-rate DMA (16 KB/partition contiguous).
- 4 GEMM chunks, each AllGathered as soon as its tiles finish; AG_0/1
  output into two halves of one shared buffer (h_gA) so messages sourced
  from chunks {0,1} gather from a single table.  Message groups:
  A={chunk0,1}, B={chunk2}, C={chunk3} -> block padding stays ~12%.
- gathers are prepare_only SWDGE ops across 4 queues, descriptors built on
  the idle Pool engine during the GEMM; trigger_dma fires each batch when
  its AllGather lands.  Strict per-queue prep/trigger alternation (a
  trigger fires *all* pending preps of its queue).  AG triggers are woven
  at the natural stall boundaries so the collective stream runs
  back-to-back.  Manual completion semaphores gate consumer matmuls.
- per (group, tile) blocks of 128 messages are routed with host-built 0/1
  one-hot matrices on the tensor engine; per-tile partials accumulate in
  SBUF bf16.
- epilogue without max-subtraction (|z| <= ~5): z = partial*dis [+bias],
  Exp with accumulate (one table load), one Ln over all tile sums, then
  res = partial*dis - lse [+bias] in bf16.  Host converts to f32.
"""

import numpy as np
import ml_dtypes

import concourse.bass as bass
import concourse.tile as tile
from concourse import bacc, mybir
from concourse.bass_utils import run_bass_kernel_spmd

bf16 = ml_dtypes.bfloat16
F32 = mybir.dt.float32
BF16 = mybir.dt.bfloat16
I16 = mybir.dt.int16

N_NODES = 50000
D_IN = 2048
D_OUT = 512
C = 8
NLOC = N_NODES // C         # 6250
T = 49
NPAD = T * 128              # 6272
KT = D_IN // 128            # 16
MG = 3                      # message/AG groups: local rows {0,1}, {2}, {3}
MG_OFF = [0, 3072, 4608]    # local row offset of each group
MG_SZ = [3072, 1536, 1664]  # local rows per group
MG_ROWS = [8 * sz for sz in MG_SZ]   # rows in each gather table (< 32768)
GB = 8                      # blocks per gather group
NQ = 4                      # SWDGE queues
NSEM = 16
SLAB = 4                    # GEMM tiles per xk slab
NSLAB = (T + SLAB - 1) // SLAB        # 13
SLABW = SLAB * 128          # 512
XCOLS = NSLAB * KT * SLABW  # pre-tiled xT columns

LAST_RESULTS = None


def _wrap_idx(idx):
    n = idx.shape[0]
    assert n % 16 == 0
    cols = n // 16
    w = np.empty((128, cols), np.int16)
    blk = idx.reshape(cols, 16).T.astype(np.int16)
    for g in range(8):
        w[g * 16:(g + 1) * 16, :] = blk
    return w


def _preprocess(x, edge_index, weight, bias):
    src = np.asarray(edge_index[0], dtype=np.int64)
    dst = np.asarray(edge_index[1], dtype=np.int64)
    loops = np.arange(N_NODES, dtype=np.int64)
    msrc = np.concatenate([src, loops])
    mdst = np.concatenate([dst, loops])

    deg = np.bincount(mdst, minlength=N_NODES).astype(np.float32)
    dis = 1.0 / np.sqrt(deg)

    sc = msrc // NLOC
    sr = msrc % NLOC
    mg = np.searchsorted(np.array(MG_OFF), sr, side="right") - 1
    g = sc * np.array(MG_SZ)[mg] + (sr - np.array(MG_OFF)[mg])

    dc = mdst // NLOC
    dr = mdst % NLOC
    t = dr // 128
    dl = dr % 128

    order = np.lexsort((g, t, mg, dc))
    g, mg, t, dl, dc = g[order], mg[order], t[order], dl[order], dc[order]

    key = (dc * MG + mg) * T + t
    counts = np.bincount(key, minlength=C * MG * T).reshape(C, MG, T)
    B = (-(-counts // 128)).max(axis=0)          # [MG, T]
    assert (B.sum(axis=0) > 0).all()

    totB = int(B.sum())
    lastg = np.zeros(T, np.int64)
    firstg = np.zeros(T, np.int64)
    for tt in range(T):
        nz = np.nonzero(B[:, tt])[0]
        firstg[tt], lastg[tt] = nz[0], nz[-1]

    starts = np.zeros(C * MG * T + 1, np.int64)
    np.cumsum(np.bincount(key, minlength=C * MG * T), out=starts[1:])

    bbase = np.zeros((MG, T), np.int64)
    bb = 0
    for gg in range(MG):
        for tt in range(T):
            bbase[gg, tt] = bb
            bb += int(B[gg, tt])

    idx_cols = 8 * totB
    idx_np = np.zeros((C, 128, idx_cols), np.int16)
    oh_np = np.zeros((C, 128, totB * 128), bf16)

    for c in range(C):
        gidx = np.zeros(totB * 128, np.int64)
        for gg in range(MG):
            for tt in range(T):
                nb = int(B[gg, tt])
                if nb == 0:
                    continue
                s0 = starts[(c * MG + gg) * T + tt]
                s1 = starts[(c * MG + gg) * T + tt + 1]
                cnt = int(s1 - s0)
                pp = np.arange(cnt)
                base = int(bbase[gg, tt])
                gidx[base * 128 + pp] = g[s0:s1]
                oh_np[c, pp % 128, (base + pp // 128) * 128 + dl[s0:s1]] = 1.0
        for gg in range(MG):
            b0 = int(bbase[gg, 0])
            bn = int(B[gg].sum())
            gs = b0
            while gs < b0 + bn:
                nb = min(GB, b0 + bn - gs)
                idx_np[c, :, 8 * gs:8 * (gs + nb)] = _wrap_idx(
                    gidx[gs * 128:(gs + nb) * 128])
                gs += nb

    # pre-tiled xT: [128, NSLAB*KT*SLABW], slab-major, k-chunk, column
    xtile = np.zeros((C, 128, XCOLS), bf16)
    dis_np = np.zeros((C, 128, T), np.float32)
    for c in range(C):
        xs = (x[c * NLOC:(c + 1) * NLOC] *
              dis[c * NLOC:(c + 1) * NLOC, None]).T.astype(bf16)  # [2048, 6250]
        arr = np.zeros((KT, 128, NSLAB * SLABW), bf16)
        arr[:, :, :NLOC] = xs.reshape(KT, 128, NLOC)
        xtile[c] = np.transpose(arr.reshape(KT, 128, NSLAB, SLABW),
                                (1, 2, 0, 3)).reshape(128, XCOLS)
        dis_np[c, :, :] = np.pad(dis[c * NLOC:(c + 1) * NLOC],
                                 (0, NPAD - NLOC)).reshape(T, 128).T

    w_bf = np.ascontiguousarray(weight.astype(bf16))
    bias_full = np.tile(np.asarray(bias, np.float32)[None, :], (128, 1))
    has_bias = bool(np.any(np.asarray(bias) != 0))

    return dict(
        B=B, firstg=firstg, lastg=lastg, bbase=bbase, has_bias=has_bias,
        idx=idx_np, oh=oh_np, w=w_bf, xtile=xtile,
        dis=dis_np, bias=np.ascontiguousarray(bias_full),
    )


def _build(B, firstg, lastg, bbase, has_bias):
    totB = int(B.sum())
    idx_cols = 8 * totB

    nc = bacc.Bacc("TRN2", target_bir_lowering=False, debug=False,
                   num_devices=C, num_swdge_queues=NQ)

    xT_t = nc.dram_tensor("xT", [128, XCOLS], BF16, kind="ExternalInput")
    w_t = nc.dram_tensor("w", [D_IN, D_OUT], BF16, kind="ExternalInput")
    dis_t = nc.dram_tensor("dis", [128, T], F32, kind="ExternalInput")
    bias_t = nc.dram_tensor("biasf", [128, D_OUT], F32, kind="ExternalInput")
    idx_t = nc.dram_tensor("idx", [128, idx_cols], I16, kind="ExternalInput")
    oh_t = nc.dram_tensor("oh", [128, totB * 128], BF16, kind="ExternalInput")
    out_t = nc.dram_tensor("out", [NPAD, D_OUT], BF16, kind="ExternalOutput")

    xT, w, dis, biasf, idx, oh, out = (
        t.ap() for t in (xT_t, w_t, dis_t, bias_t, idx_t, oh_t, out_t))

    # gather groups per message-group
    groups = []                    # gi -> (mg, gstart, nb)
    grp_of_block = {}
    mg_groups = [[] for _ in range(MG)]
    for gg in range(MG):
        b0 = int(bbase[gg, 0])
        bn = int(B[gg].sum())
        gs = b0
        while gs < b0 + bn:
            nb = min(GB, b0 + bn - gs)
            gi = len(groups)
            groups.append((gg, gs, nb))
            mg_groups[gg].append(gi)
            for b in range(gs, gs + nb):
                grp_of_block[b] = gi
            gs += nb

    # batches of NQ groups (one per queue)
    def make_batches(gis):
        return [gis[i:i + NQ] for i in range(0, len(gis), NQ)]

    mg_batches = [make_batches(mg_groups[gg]) for gg in range(MG)]

    with tile.TileContext(nc) as tc:
        with tc.tile_pool(name="const", bufs=1) as constp, \
             tc.tile_pool(name="xk", bufs=2) as xkp, \
             tc.tile_pool(name="hc", bufs=3) as hcp, \
             tc.tile_pool(name="gath", bufs=8) as gp, \
             tc.tile_pool(name="ohp", bufs=6) as ohp, \
             tc.tile_pool(name="zt", bufs=2) as zp, \
             tc.tile_pool(name="res", bufs=2) as resp, \
             tc.tile_pool(name="gps", bufs=3, space="PSUM") as gpsp, \
             tc.tile_pool(name="aps", bufs=3, space="PSUM") as apsp, \
             tc.tile_pool(name="eps", bufs=2, space="PSUM") as epsp, \
             tc.tile_pool(name="dram", bufs=1, space="DRAM") as dramp:

            # ---- resident constants ----
            w_sb = constp.tile([128, KT, D_OUT], BF16)
            for kt in range(KT):
                nc.sync.dma_start(out=w_sb[:, kt, :],
                                  in_=w[kt * 128:(kt + 1) * 128, :])
            dis_sb = constp.tile([128, T], F32)
            nc.sync.dma_start(out=dis_sb[:], in_=dis[:])
            bias_sb = constp.tile([128, D_OUT], F32)
            nc.sync.dma_start(out=bias_sb[:], in_=biasf[:])
            idx_sb = constp.tile([128, idx_cols], I16)
            nc.sync.dma_start(out=idx_sb[:], in_=idx[:])

            partial = constp.tile([128, T, D_OUT], BF16)
            sums = constp.tile([128, T], F32)
            lse = constp.tile([128, T], F32)
            nlse = constp.tile([128, T], F32)

            h_loc = [dramp.tile([MG_SZ[gg], D_OUT], BF16, name=f"h_loc{gg}")
                     for gg in range(MG)]
            h_tab = [dramp.tile([MG_ROWS[gg], D_OUT], BF16,
                                addr_space="Shared", name=f"h_tab{gg}")
                     for gg in range(MG)]

            SPQ = NSEM // NQ
            sems = [[nc.alloc_semaphore(f"gsem{q}_{i}") for i in range(SPQ)]
                    for q in range(NQ)]
            sem_uses = [[0] * SPQ for _ in range(NQ)]
            sem_rr = [0] * NQ
            grp_wait = {}

            # ---- phase 1: GEMM ----
            for s in range(NSLAB):
                t0 = s * SLAB
                nt = min(SLAB, T - t0)
                xk = xkp.tile([128, KT, SLABW], BF16, name="xk")
                nc.sync.dma_start(
                    out=xk[:, :, :],
                    in_=xT[:, s * KT * SLABW:(s + 1) * KT * SLABW])
                for ti in range(nt):
                    t_ = t0 + ti
                    ph = gpsp.tile([128, D_OUT], F32, name="ph")
                    for kt in range(KT):
                        nc.tensor.matmul(
                            ph[:], xk[:, kt, ti * 128:(ti + 1) * 128],
                            w_sb[:, kt, :], start=(kt == 0), stop=(kt == KT - 1))
                    hcv = hcp.tile([128, D_OUT], BF16, name="hcv")
                    nc.vector.tensor_copy(hcv[:], ph[:])
                    kk = next(i for i in range(MG)
                              if MG_OFF[i] <= t_ * 128 < MG_OFF[i] + MG_SZ[i])
                    r0 = t_ * 128 - MG_OFF[kk]
                    nc.sync.dma_start(out=h_loc[kk][r0:r0 + 128, :],
                                      in_=hcv[:])

            # ---- phase 2: Pool program ----
            rg = [list(range(C))]

            def emit_ag(kk):
                nc.gpsimd.collective_compute(
                    "AllGather", mybir.AluOpType.bypass,
                    replica_groups=rg,
                    ins=[h_loc[kk].opt()], outs=[h_tab[kk].opt()])

            gtiles = {}

            def emit_prep_batch(gis):
                for j, gi in enumerate(gis):
                    gg, gs, nb = groups[gi]
                    gt = gp.tile([128, GB, D_OUT], BF16, name="gt", tag="gt")
                    gtiles[gi] = gt
                    q = j % NQ
                    si = sem_rr[q] % SPQ
                    sem_rr[q] += 1
                    sem_uses[q][si] += 1
                    grp_wait[gi] = (sems[q][si], 16 * sem_uses[q][si])
                    nc.gpsimd.dma_gather(
                        out_ap=gt[:, :nb, :], in_ap=h_tab[gg][:],
                        idxs_ap=idx_sb[:, 8 * gs:8 * (gs + nb)],
                        num_idxs=nb * 128, num_idxs_reg=nb * 128,
                        elem_size=D_OUT, prepare_only=True, sem=sems[q][si],
                        queue_num=q)

            def emit_trigger_batch(gis):
                for j in range(min(len(gis), NQ)):
                    nc.gpsimd.trigger_dma(count=None, queue_num=j)

            # strict per-queue prep/trigger alternation; AG triggers woven in
            emit_ag(0)
            emit_prep_batch(mg_batches[0][0])
            emit_ag(1)
            emit_ag(2)
            emit_trigger_batch(mg_batches[0][0])
            for pb in mg_batches[0][1:]:
                emit_prep_batch(pb)
                emit_trigger_batch(pb)
            for gg in (1, 2):
                for pb in mg_batches[gg]:
                    emit_prep_batch(pb)
                    emit_trigger_batch(pb)

            # ---- phase 3: aggregation + epilogue sweep 1 ----
            _oh_tiles = {}
            waited = set()
            for gg in range(MG):
                for tt in range(T):
                    nb_t = int(B[gg, tt])
                    if nb_t == 0:
                        continue
                    ps = apsp.tile([128, D_OUT], F32, name="ps")
                    base = int(bbase[gg, tt])
                    for b in range(nb_t):
                        bi_ = base + b
                        gi = grp_of_block[bi_]
                        if gi not in waited:
                            sem, val = grp_wait[gi]
                            nc.tensor.wait_ge(sem, val)
                            waited.add(gi)
                        _, gs, nb = groups[gi]
                        slot = bi_ - gs
                        oht = _oh_tiles.get(gi)
                        if oht is None:
                            oht = ohp.tile([128, GB * 128], BF16, name="oht")
                            nc.scalar.dma_start(
                                out=oht[:, :nb * 128],
                                in_=oh[:, gs * 128:(gs + nb) * 128])
                            _oh_tiles[gi] = oht
                        nc.tensor.matmul(
                            ps[:], oht[:, slot * 128:(slot + 1) * 128],
                            gtiles[gi][:, slot, :],
                            start=(b == 0), stop=(b == nb_t - 1))
                    if gg == firstg[tt]:
                        nc.vector.tensor_copy(partial[:, tt, :], ps[:])
                    else:
                        nc.vector.tensor_tensor(
                            partial[:, tt, :], partial[:, tt, :], ps[:],
                            mybir.AluOpType.add)
                    if gg == lastg[tt]:
                        zt = zp.tile([128, D_OUT], F32, name="zt")
                        nc.vector.tensor_scalar(
                            zt[:], partial[:, tt, :], dis_sb[:, tt:tt + 1],
                            None, mybir.AluOpType.mult)
                        if has_bias:
                            nc.vector.tensor_tensor(
                                zt[:], zt[:], bias_sb[:], mybir.AluOpType.add)
                        ep = epsp.tile([128, D_OUT], F32, name="ep")
                        nc.scalar.activation(
                            ep[:], zt[:], mybir.ActivationFunctionType.Exp,
                            accum_out=sums[:, tt:tt + 1])

            # ---- phase 4: lse + final sweep ----
            nc.scalar.activation(lse[:], sums[:],
                                 mybir.ActivationFunctionType.Ln)
            nc.vector.tensor_scalar(nlse[:], lse[:], -1.0, None,
                                    mybir.AluOpType.mult)
            for tt in range(T):
                res = resp.tile([128, D_OUT], BF16, name="res")
                if has_bias:
                    zt = zp.tile([128, D_OUT], F32, name="zt2")
                    nc.vector.tensor_scalar(
                        zt[:], partial[:, tt, :], dis_sb[:, tt:tt + 1],
                        nlse[:, tt:tt + 1], mybir.AluOpType.mult,
                        mybir.AluOpType.add)
                    nc.vector.tensor_tensor(res[:], zt[:], bias_sb[:],
                                            mybir.AluOpType.add)
                else:
                    nc.vector.tensor_scalar(
                        res[:], partial[:, tt, :], dis_sb[:, tt:tt + 1],
                        nlse[:, tt:tt + 1], mybir.AluOpType.mult,
                        mybir.AluOpType.add)
                nc.scalar.dma_start(out=out[tt * 128:(tt + 1) * 128, :],
                                    in_=res[:])

    nc.compile()
    return nc


def kernel(x, edge_index, weight, bias):
    global LAST_RESULTS
    x = np.asarray(x, dtype=np.float32)
    weight = np.asarray(weight, dtype=np.float32)
    bias = np.asarray(bias, dtype=np.float32)

    pp = _preprocess(x, edge_index, weight, bias)
    nc = _build(pp["B"], pp["firstg"], pp["lastg"], pp["bbase"],
                pp["has_bias"])

    in_maps = []
    for c in range(C):
        in_maps.append({
            "xT": np.ascontiguousarray(pp["xtile"][c]),
            "w": pp["w"],
            "dis": np.ascontiguousarray(pp["dis"][c]),
            "biasf": pp["bias"],
            "idx": np.ascontiguousarray(pp["idx"][c]),
            "oh": np.ascontiguousarray(pp["oh"][c]),
        })

    res = run_bass_kernel_spmd(nc, in_maps, core_ids=list(range(C)))
    LAST_RESULTS = res

    out = np.empty((N_NODES, D_OUT), np.float32)
    for c in range(C):
        out[c * NLOC:(c + 1) * NLOC] = \
            res.results[c]["out"][:NLOC].astype(np.float32)
    return out


# revision 13
# speedup vs baseline: 1.1228x; 1.0061x over previous
"""GCN layer (GCNConv + log_softmax) on 8 Trainium2 NeuronCores.

v3 design:
- nodes row-sharded 8 ways; h' = (dis*x) @ W in bf16 (dis folded into x on
  host).  xT is pre-tiled on host into the exact SBUF slab layout so each
  slab loads with one full-rate DMA (16 KB/partition contiguous).
- 4 GEMM chunks, each AllGathered as soon as its tiles finish; AG_0/1
  output into two halves of one shared buffer (h_gA) so messages sourced
  from chunks {0,1} gather from a single table.  Message groups:
  A={chunk0,1}, B={chunk2}, C={chunk3} -> block padding stays ~12%.
- gathers are prepare_only SWDGE ops across 4 queues, descriptors built on
  the idle Pool engine during the GEMM; trigger_dma fires each batch when
  its AllGather lands.  Strict per-queue prep/trigger alternation (a
  trigger fires *all* pending preps of its queue).  AG triggers are woven
  at the natural stall boundaries so the collective stream runs
  back-to-back.  Manual completion semaphores gate consumer matmuls.
- per (group, tile) blocks of 128 messages are routed with host-built 0/1
  one-hot matrices on the tensor engine; per-tile partials accumulate in
  SBUF bf16.
- epilogue without max-subtraction (|z| <= ~5): z = partial*dis [+bias],
  Exp with accumulate (one table load), one Ln over all tile sums, then
  res = partial*dis - lse [+bias] in bf16.  Host converts to f32.
"""

import numpy as np
import ml_dtypes

import concourse.bass as bass
import concourse.tile as tile
from concourse import bacc, mybir
from concourse.bass_utils import run_bass_kernel_spmd

bf16 = ml_dtypes.bfloat16
F32 = mybir.dt.float32
BF16 = mybir.dt.bfloat16
I16 = mybir.dt.int16

N_NODES = 50000
D_IN = 2048
D_OUT = 512
C = 8
NLOC = N_NODES // C         # 6250
T = 49
NPAD = T * 128              # 6272
KT = D_IN // 128            # 16
MG = 3                      # message/AG groups: local rows {0,1}, {2}, {3}
MG_OFF = [0, 3072, 4608]    # local row offset of each group
MG_SZ = [3072, 1536, 1664]  # local rows per group
MG_ROWS = [8 * sz for sz in MG_SZ]   # rows in each gather table (< 32768)
GB = 8                      # blocks per gather group
NQ = 4                      # SWDGE queues
NSEM = 16
SLAB = 4                    # GEMM tiles per xk slab
NSLAB = (T + SLAB - 1) // SLAB        # 13
SLABW = SLAB * 128          # 512
XCOLS = NSLAB * KT * SLABW  # pre-tiled xT columns

LAST_RESULTS = None


def _wrap_idx(idx):
    n = idx.shape[0]
    assert n % 16 == 0
    cols = n // 16
    w = np.empty((128, cols), np.int16)
    blk = idx.reshape(cols, 16).T.astype(np.int16)
    for g in range(8):
        w[g * 16:(g + 1) * 16, :] = blk
    return w


def _preprocess(x, edge_index, weight, bias):
    src = np.asarray(edge_index[0], dtype=np.int64)
    dst = np.asarray(edge_index[1], dtype=np.int64)
    loops = np.arange(N_NODES, dtype=np.int64)
    msrc = np.concatenate([src, loops])
    mdst = np.concatenate([dst, loops])

    deg = np.bincount(mdst, minlength=N_NODES).astype(np.float32)
    dis = 1.0 / np.sqrt(deg)

    sc = msrc // NLOC
    sr = msrc % NLOC
    mg = np.searchsorted(np.array(MG_OFF), sr, side="right") - 1
    g = sc * np.array(MG_SZ)[mg] + (sr - np.array(MG_OFF)[mg])

    dc = mdst // NLOC
    dr = mdst % NLOC
    t = dr // 128
    dl = dr % 128

    order = np.lexsort((g, t, mg, dc))
    g, mg, t, dl, dc = g[order], mg[order], t[order], dl[order], dc[order]

    key = (dc * MG + mg) * T + t
    counts = np.bincount(key, minlength=C * MG * T).reshape(C, MG, T)
    B = (-(-counts // 128)).max(axis=0)          # [MG, T]
    assert (B.sum(axis=0) > 0).all()

    totB = int(B.sum())
    lastg = np.zeros(T, np.int64)
    firstg = np.zeros(T, np.int64)
    for tt in range(T):
        nz = np.nonzero(B[:, tt])[0]
        firstg[tt], lastg[tt] = nz[0], nz[-1]

    starts = np.zeros(C * MG * T + 1, np.int64)
    np.cumsum(np.bincount(key, minlength=C * MG * T), out=starts[1:])

    bbase = np.zeros((MG, T), np.int64)
    bb = 0
    for gg in range(MG):
        for tt in range(T):
            bbase[gg, tt] = bb
            bb += int(B[gg, tt])

    idx_cols = 8 * totB
    idx_np = np.zeros((C, 128, idx_cols), np.int16)
    oh_np = np.zeros((C, 128, totB * 128), bf16)

    for c in range(C):
        gidx = np.zeros(totB * 128, np.int64)
        for gg in range(MG):
            for tt in range(T):
                nb = int(B[gg, tt])
                if nb == 0:
                    continue
                s0 = starts[(c * MG + gg) * T + tt]
                s1 = starts[(c * MG + gg) * T + tt + 1]
                cnt = int(s1 - s0)
                pp = np.arange(cnt)
                base = int(bbase[gg, tt])
                gidx[base * 128 + pp] = g[s0:s1]
                oh_np[c, pp % 128, (base + pp // 128) * 128 + dl[s0:s1]] = 1.0
        for gg in range(MG):
            b0 = int(bbase[gg, 0])
            bn = int(B[gg].sum())
            gs = b0
            while gs < b0 + bn:
                nb = min(GB, b0 + bn - gs)
                idx_np[c, :, 8 * gs:8 * (gs + nb)] = _wrap_idx(
                    gidx[gs * 128:(gs + nb) * 128])
                gs += nb

    # pre-tiled xT: [128, NSLAB*KT*SLABW], slab-major, k-chunk, column
    xtile = np.zeros((C, 128, XCOLS), bf16)
    dis_np = np.zeros((C, 128, T), np.float32)
    for c in range(C):
        xs = (x[c * NLOC:(c + 1) * NLOC] *
              dis[c * NLOC:(c + 1) * NLOC, None]).T.astype(bf16)  # [2048, 6250]
        arr = np.zeros((KT, 128, NSLAB * SLABW), bf16)
        arr[:, :, :NLOC] = xs.reshape(KT, 128, NLOC)
        xtile[c] = np.transpose(arr.reshape(KT, 128, NSLAB, SLABW),
                                (1, 2, 0, 3)).reshape(128, XCOLS)
        dis_np[c, :, :] = np.pad(dis[c * NLOC:(c + 1) * NLOC],
                                 (0, NPAD - NLOC)).reshape(T, 128).T

    w_bf = np.ascontiguousarray(weight.astype(bf16))
    bias_full = np.tile(np.asarray(bias, np.float32)[None, :], (128, 1))
    has_bias = bool(np.any(np.asarray(bias) != 0))

    return dict(
        B=B, firstg=firstg, lastg=lastg, bbase=bbase, has_bias=has_bias,
        idx=idx_np, oh=oh_np, w=w_bf, xtile=xtile,
        dis=dis_np, bias=np.ascontiguousarray(bias_full),
    )


def _build(B, firstg, lastg, bbase, has_bias):
    totB = int(B.sum())
    idx_cols = 8 * totB

    nc = bacc.Bacc("TRN2", target_bir_lowering=False, debug=False,
                   num_devices=C, num_swdge_queues=NQ)

    xT_t = nc.dram_tensor("xT", [128, XCOLS], BF16, kind="ExternalInput")
    w_t = nc.dram_tensor("w", [D_IN, D_OUT], BF16, kind="ExternalInput")
    dis_t = nc.dram_tensor("dis", [128, T], F32, kind="ExternalInput")
    bias_t = nc.dram_tensor("biasf", [128, D_OUT], F32, kind="ExternalInput")
    idx_t = nc.dram_tensor("idx", [128, idx_cols], I16, kind="ExternalInput")
    oh_t = nc.dram_tensor("oh", [128, totB * 128], BF16, kind="ExternalInput")
    out_t = nc.dram_tensor("out", [NPAD, D_OUT], BF16, kind="ExternalOutput")

    xT, w, dis, biasf, idx, oh, out = (
        t.ap() for t in (xT_t, w_t, dis_t, bias_t, idx_t, oh_t, out_t))

    # gather groups per message-group
    groups = []                    # gi -> (mg, gstart, nb)
    grp_of_block = {}
    mg_groups = [[] for _ in range(MG)]
    for gg in range(MG):
        b0 = int(bbase[gg, 0])
        bn = int(B[gg].sum())
        gs = b0
        while gs < b0 + bn:
            nb = min(GB, b0 + bn - gs)
            gi = len(groups)
            groups.append((gg, gs, nb))
            mg_groups[gg].append(gi)
            for b in range(gs, gs + nb):
                grp_of_block[b] = gi
            gs += nb

    # batches of NQ groups (one per queue)
    def make_batches(gis):
        return [gis[i:i + NQ] for i in range(0, len(gis), NQ)]

    mg_batches = [make_batches(mg_groups[gg]) for gg in range(MG)]

    with tile.TileContext(nc) as tc:
        with tc.tile_pool(name="const", bufs=1) as constp, \
             tc.tile_pool(name="xk", bufs=2) as xkp, \
             tc.tile_pool(name="hc", bufs=3) as hcp, \
             tc.tile_pool(name="gath", bufs=8) as gp, \
             tc.tile_pool(name="ohp", bufs=6) as ohp, \
             tc.tile_pool(name="zt", bufs=2) as zp, \
             tc.tile_pool(name="res", bufs=2) as resp, \
             tc.tile_pool(name="gps", bufs=3, space="PSUM") as gpsp, \
             tc.tile_pool(name="aps", bufs=3, space="PSUM") as apsp, \
             tc.tile_pool(name="eps", bufs=2, space="PSUM") as epsp, \
             tc.tile_pool(name="dram", bufs=1, space="DRAM") as dramp:

            # ---- resident constants ----
            w_sb = constp.tile([128, KT, D_OUT], BF16)
            for kt in range(KT):
                nc.sync.dma_start(out=w_sb[:, kt, :],
                                  in_=w[kt * 128:(kt + 1) * 128, :])
            dis_sb = constp.tile([128, T], F32)
            nc.scalar.dma_start(out=dis_sb[:], in_=dis[:])
            bias_sb = constp.tile([128, D_OUT], F32)
            nc.scalar.dma_start(out=bias_sb[:], in_=biasf[:])
            idx_sb = constp.tile([128, idx_cols], I16)
            nc.scalar.dma_start(out=idx_sb[:], in_=idx[:])

            partial = constp.tile([128, T, D_OUT], BF16)
            sums = constp.tile([128, T], F32)
            lse = constp.tile([128, T], F32)
            nlse = constp.tile([128, T], F32)

            h_loc = [dramp.tile([MG_SZ[gg], D_OUT], BF16, name=f"h_loc{gg}")
                     for gg in range(MG)]
            h_tab = [dramp.tile([MG_ROWS[gg], D_OUT], BF16,
                                addr_space="Shared", name=f"h_tab{gg}")
                     for gg in range(MG)]

            SPQ = NSEM // NQ
            sems = [[nc.alloc_semaphore(f"gsem{q}_{i}") for i in range(SPQ)]
                    for q in range(NQ)]
            sem_uses = [[0] * SPQ for _ in range(NQ)]
            sem_rr = [0] * NQ
            grp_wait = {}

            # ---- phase 1: GEMM ----
            for s in range(NSLAB):
                t0 = s * SLAB
                nt = min(SLAB, T - t0)
                xk = xkp.tile([128, KT, SLABW], BF16, name="xk")
                nc.sync.dma_start(
                    out=xk[:, :, :],
                    in_=xT[:, s * KT * SLABW:(s + 1) * KT * SLABW])
                for ti in range(nt):
                    t_ = t0 + ti
                    ph = gpsp.tile([128, D_OUT], F32, name="ph")
                    for kt in range(KT):
                        nc.tensor.matmul(
                            ph[:], xk[:, kt, ti * 128:(ti + 1) * 128],
                            w_sb[:, kt, :], start=(kt == 0), stop=(kt == KT - 1))
                    hcv = hcp.tile([128, D_OUT], BF16, name="hcv")
                    nc.vector.tensor_copy(hcv[:], ph[:])
                    kk = next(i for i in range(MG)
                              if MG_OFF[i] <= t_ * 128 < MG_OFF[i] + MG_SZ[i])
                    r0 = t_ * 128 - MG_OFF[kk]
                    nc.sync.dma_start(out=h_loc[kk][r0:r0 + 128, :],
                                      in_=hcv[:])

            # ---- phase 2: Pool program ----
            rg = [list(range(C))]

            def emit_ag(kk):
                nc.gpsimd.collective_compute(
                    "AllGather", mybir.AluOpType.bypass,
                    replica_groups=rg,
                    ins=[h_loc[kk].opt()], outs=[h_tab[kk].opt()])

            gtiles = {}

            def emit_prep_batch(gis):
                for j, gi in enumerate(gis):
                    gg, gs, nb = groups[gi]
                    gt = gp.tile([128, GB, D_OUT], BF16, name="gt", tag="gt")
                    gtiles[gi] = gt
                    q = j % NQ
                    si = sem_rr[q] % SPQ
                    sem_rr[q] += 1
                    sem_uses[q][si] += 1
                    grp_wait[gi] = (sems[q][si], 16 * sem_uses[q][si])
                    nc.gpsimd.dma_gather(
                        out_ap=gt[:, :nb, :], in_ap=h_tab[gg][:],
                        idxs_ap=idx_sb[:, 8 * gs:8 * (gs + nb)],
                        num_idxs=nb * 128, num_idxs_reg=nb * 128,
                        elem_size=D_OUT, prepare_only=True, sem=sems[q][si],
                        single_packet=False, queue_num=q)

            def emit_trigger_batch(gis):
                for j in range(min(len(gis), NQ)):
                    nc.gpsimd.trigger_dma(count=None, queue_num=j)

            # strict per-queue prep/trigger alternation; AG triggers woven in
            emit_ag(0)
            emit_prep_batch(mg_batches[0][0])
            emit_ag(1)
            emit_ag(2)
            emit_trigger_batch(mg_batches[0][0])
            for pb in mg_batches[0][1:]:
                emit_prep_batch(pb)
                emit_trigger_batch(pb)
            for gg in (1, 2):
                for pb in mg_batches[gg]:
                    emit_prep_batch(pb)
                    emit_trigger_batch(pb)

            # ---- phase 3: aggregation + epilogue sweep 1 ----
            _oh_tiles = {}
            waited = set()
            for gg in range(MG):
                for tt in range(T):
                    nb_t = int(B[gg, tt])
                    if nb_t == 0:
                        continue
                    ps = apsp.tile([128, D_OUT], F32, name="ps")
                    base = int(bbase[gg, tt])
                    for b in range(nb_t):
                        bi_ = base + b
                        gi = grp_of_block[bi_]
                        if gi not in waited:
                            sem, val = grp_wait[gi]
                            nc.tensor.wait_ge(sem, val)
                            waited.add(gi)
                        _, gs, nb = groups[gi]
                        slot = bi_ - gs
                        oht = _oh_tiles.get(gi)
                        if oht is None:
                            oht = ohp.tile([128, GB * 128], BF16, name="oht")
                            nc.scalar.dma_start(
                                out=oht[:, :nb * 128],
                                in_=oh[:, gs * 128:(gs + nb) * 128])
                            _oh_tiles[gi] = oht
                        nc.tensor.matmul(
                            ps[:], oht[:, slot * 128:(slot + 1) * 128],
                            gtiles[gi][:, slot, :],
                            start=(b == 0), stop=(b == nb_t - 1))
                    if gg == firstg[tt]:
                        nc.vector.tensor_copy(partial[:, tt, :], ps[:])
                    else:
                        nc.vector.tensor_tensor(
                            partial[:, tt, :], partial[:, tt, :], ps[:],
                            mybir.AluOpType.add)
                    if gg == lastg[tt]:
                        zt = zp.tile([128, D_OUT], F32, name="zt")
                        nc.vector.tensor_scalar(
                            zt[:], partial[:, tt, :], dis_sb[:, tt:tt + 1],
                            None, mybir.AluOpType.mult)
                        if has_bias:
                            nc.vector.tensor_tensor(
                                zt[:], zt[:], bias_sb[:], mybir.AluOpType.add)
                        ep = epsp.tile([128, D_OUT], F32, name="ep")
                        nc.scalar.activation(
                            ep[:], zt[:], mybir.ActivationFunctionType.Exp,
                            accum_out=sums[:, tt:tt + 1])

            # ---- phase 4: lse + final sweep ----
            nc.scalar.activation(lse[:], sums[:],
                                 mybir.ActivationFunctionType.Ln)
            nc.vector.tensor_scalar(nlse[:], lse[:], -1.0, None,
                                    mybir.AluOpType.mult)
            for tt in range(T):
                res = resp.tile([128, D_OUT], BF16, name="res")
                if has_bias:
                    zt = zp.tile([128, D_OUT], F32, name="zt2")
                    nc.vector.tensor_scalar(
                        zt[:], partial[:, tt, :], dis_sb[:, tt:tt + 1],
                        nlse[:, tt:tt + 1], mybir.AluOpType.mult,
                        mybir.AluOpType.add)
                    nc.vector.tensor_tensor(res[:], zt[:], bias_sb[:],
                                            mybir.AluOpType.add)
                else:
                    nc.vector.tensor_scalar(
                        res[:], partial[:, tt, :], dis_sb[:, tt:tt + 1],
                        nlse[:, tt:tt + 1], mybir.AluOpType.mult,
                        mybir.AluOpType.add)
                nc.scalar.dma_start(out=out[tt * 128:(tt + 1) * 128, :],
                                    in_=res[:])

    nc.compile()
    return nc


def kernel(x, edge_index, weight, bias):
    global LAST_RESULTS
    x = np.asarray(x, dtype=np.float32)
    weight = np.asarray(weight, dtype=np.float32)
    bias = np.asarray(bias, dtype=np.float32)

    pp = _preprocess(x, edge_index, weight, bias)
    nc = _build(pp["B"], pp["firstg"], pp["lastg"], pp["bbase"],
                pp["has_bias"])

    in_maps = []
    for c in range(C):
        in_maps.append({
            "xT": np.ascontiguousarray(pp["xtile"][c]),
            "w": pp["w"],
            "dis": np.ascontiguousarray(pp["dis"][c]),
            "biasf": pp["bias"],
            "idx": np.ascontiguousarray(pp["idx"][c]),
            "oh": np.ascontiguousarray(pp["oh"][c]),
        })

    res = run_bass_kernel_spmd(nc, in_maps, core_ids=list(range(C)))
    LAST_RESULTS = res

    out = np.empty((N_NODES, D_OUT), np.float32)
    for c in range(C):
        out[c * NLOC:(c + 1) * NLOC] = \
            res.results[c]["out"][:NLOC].astype(np.float32)
    return out


# revision 16
# speedup vs baseline: 1.1647x; 1.0373x over previous
"""GCN layer (GCNConv + log_softmax) on 8 Trainium2 NeuronCores.

v3 design:
- nodes row-sharded 8 ways; h' = (dis*x) @ W in bf16 (dis folded into x on
  host).  xT is pre-tiled on host into the exact SBUF slab layout so each
  slab loads with one full-rate DMA (16 KB/partition contiguous).
- 4 GEMM chunks, each AllGathered as soon as its tiles finish; AG_0/1
  output into two halves of one shared buffer (h_gA) so messages sourced
  from chunks {0,1} gather from a single table.  Message groups:
  A={chunk0,1}, B={chunk2}, C={chunk3} -> block padding stays ~12%.
- gathers are prepare_only SWDGE ops across 4 queues, descriptors built on
  the idle Pool engine during the GEMM; trigger_dma fires each batch when
  its AllGather lands.  Strict per-queue prep/trigger alternation (a
  trigger fires *all* pending preps of its queue).  AG triggers are woven
  at the natural stall boundaries so the collective stream runs
  back-to-back.  Manual completion semaphores gate consumer matmuls.
- per (group, tile) blocks of 128 messages are routed with host-built 0/1
  one-hot matrices on the tensor engine; per-tile partials accumulate in
  SBUF bf16.
- epilogue without max-subtraction (|z| <= ~5): z = partial*dis [+bias],
  Exp with accumulate (one table load), one Ln over all tile sums, then
  res = partial*dis - lse [+bias] in bf16.  Host converts to f32.
"""

import numpy as np
import ml_dtypes

import concourse.bass as bass
import concourse.tile as tile
from concourse import bacc, mybir
from concourse.bass_utils import run_bass_kernel_spmd

bf16 = ml_dtypes.bfloat16
fp8 = ml_dtypes.float8_e4m3
F32 = mybir.dt.float32
BF16 = mybir.dt.bfloat16
FP8 = mybir.dt.float8e4
I16 = mybir.dt.int16

N_NODES = 50000
D_IN = 2048
D_OUT = 512
C = 8
NLOC = N_NODES // C         # 6250
T = 49
NPAD = T * 128              # 6272
KT = D_IN // 128            # 16
MG = 3                      # message/AG groups: local rows {0,1}, {2}, {3}
MG_OFF = [0, 3072, 4608]    # local row offset of each group
MG_SZ = [3072, 1536, 1664]  # local rows per group
MG_ROWS = [8 * sz for sz in MG_SZ]   # rows in each gather table (< 32768)
GB = 8                      # blocks per gather group
NQ = 4                      # SWDGE queues
NSEM = 16
SLAB = 4                    # GEMM tiles per xk slab
NSLAB = (T + SLAB - 1) // SLAB        # 13
SLABW = SLAB * 128          # 512
XCOLS = NSLAB * KT * SLABW  # pre-tiled xT columns

LAST_RESULTS = None


def _wrap_idx(idx):
    n = idx.shape[0]
    assert n % 16 == 0
    cols = n // 16
    w = np.empty((128, cols), np.int16)
    blk = idx.reshape(cols, 16).T.astype(np.int16)
    for g in range(8):
        w[g * 16:(g + 1) * 16, :] = blk
    return w


def _preprocess(x, edge_index, weight, bias):
    src = np.asarray(edge_index[0], dtype=np.int64)
    dst = np.asarray(edge_index[1], dtype=np.int64)
    # degree includes the self loop; the self-loop message itself is applied
    # locally at GEMM time (partial init), not via the gather path.
    deg = (np.bincount(dst, minlength=N_NODES) + 1).astype(np.float32)
    dis = 1.0 / np.sqrt(deg)
    msrc, mdst = src, dst

    sc = msrc // NLOC
    sr = msrc % NLOC
    mg = np.searchsorted(np.array(MG_OFF), sr, side="right") - 1
    g = sc * np.array(MG_SZ)[mg] + (sr - np.array(MG_OFF)[mg])

    dc = mdst // NLOC
    dr = mdst % NLOC
    t = dr // 128
    dl = dr % 128

    order = np.lexsort((g, t, mg, dc))
    g, mg, t, dl, dc = g[order], mg[order], t[order], dl[order], dc[order]

    key = (dc * MG + mg) * T + t
    counts = np.bincount(key, minlength=C * MG * T).reshape(C, MG, T)
    B = (-(-counts // 128)).max(axis=0)          # [MG, T]
    assert (B.sum(axis=0) > 0).all()

    totB = int(B.sum())
    lastg = np.zeros(T, np.int64)
    firstg = np.zeros(T, np.int64)
    for tt in range(T):
        nz = np.nonzero(B[:, tt])[0]
        assert len(nz) > 0
        firstg[tt], lastg[tt] = nz[0], nz[-1]

    starts = np.zeros(C * MG * T + 1, np.int64)
    np.cumsum(np.bincount(key, minlength=C * MG * T), out=starts[1:])

    bbase = np.zeros((MG, T), np.int64)
    bb = 0
    for gg in range(MG):
        for tt in range(T):
            bbase[gg, tt] = bb
            bb += int(B[gg, tt])

    idx_cols = 8 * totB
    idx_np = np.zeros((C, 128, idx_cols), np.int16)
    oh_np = np.zeros((C, 128, totB * 128), fp8)

    for c in range(C):
        gidx = np.zeros(totB * 128, np.int64)
        for gg in range(MG):
            for tt in range(T):
                nb = int(B[gg, tt])
                if nb == 0:
                    continue
                s0 = starts[(c * MG + gg) * T + tt]
                s1 = starts[(c * MG + gg) * T + tt + 1]
                cnt = int(s1 - s0)
                pp = np.arange(cnt)
                base = int(bbase[gg, tt])
                gidx[base * 128 + pp] = g[s0:s1]
                oh_np[c, pp % 128, (base + pp // 128) * 128 + dl[s0:s1]] = 1.0
        for gg in range(MG):
            b0 = int(bbase[gg, 0])
            bn = int(B[gg].sum())
            gs = b0
            while gs < b0 + bn:
                nb = min(GB, b0 + bn - gs)
                idx_np[c, :, 8 * gs:8 * (gs + nb)] = _wrap_idx(
                    gidx[gs * 128:(gs + nb) * 128])
                gs += nb

    # pre-tiled xT: [128, NSLAB*KT*SLABW], slab-major, k-chunk, column
    xtile = np.zeros((C, 128, XCOLS), bf16)
    dis_np = np.zeros((C, 128, T), np.float32)
    for c in range(C):
        xs = (x[c * NLOC:(c + 1) * NLOC] *
              dis[c * NLOC:(c + 1) * NLOC, None]).T.astype(bf16)  # [2048, 6250]
        arr = np.zeros((KT, 128, NSLAB * SLABW), bf16)
        arr[:, :, :NLOC] = xs.reshape(KT, 128, NLOC)
        xtile[c] = np.transpose(arr.reshape(KT, 128, NSLAB, SLABW),
                                (1, 2, 0, 3)).reshape(128, XCOLS)
        dis_np[c, :, :] = np.pad(dis[c * NLOC:(c + 1) * NLOC],
                                 (0, NPAD - NLOC)).reshape(T, 128).T

    w_bf = np.ascontiguousarray(weight.astype(bf16))
    bias_full = np.tile(np.asarray(bias, np.float32)[None, :], (128, 1))
    has_bias = bool(np.any(np.asarray(bias) != 0))

    return dict(
        B=B, firstg=firstg, lastg=lastg, bbase=bbase, has_bias=has_bias,
        idx=idx_np, oh=oh_np, w=w_bf, xtile=xtile,
        dis=dis_np, bias=np.ascontiguousarray(bias_full),
    )


def _build(B, firstg, lastg, bbase, has_bias):
    totB = int(B.sum())
    idx_cols = 8 * totB

    nc = bacc.Bacc("TRN2", target_bir_lowering=False, debug=False,
                   num_devices=C, num_swdge_queues=NQ)

    xT_t = nc.dram_tensor("xT", [128, XCOLS], BF16, kind="ExternalInput")
    w_t = nc.dram_tensor("w", [D_IN, D_OUT], BF16, kind="ExternalInput")
    dis_t = nc.dram_tensor("dis", [128, T], F32, kind="ExternalInput")
    bias_t = nc.dram_tensor("biasf", [128, D_OUT], F32, kind="ExternalInput")
    idx_t = nc.dram_tensor("idx", [128, idx_cols], I16, kind="ExternalInput")
    oh_t = nc.dram_tensor("oh", [128, totB * 128], FP8, kind="ExternalInput")
    out_t = nc.dram_tensor("out", [NPAD, D_OUT], BF16, kind="ExternalOutput")

    xT, w, dis, biasf, idx, oh, out = (
        t.ap() for t in (xT_t, w_t, dis_t, bias_t, idx_t, oh_t, out_t))

    # gather groups per message-group
    groups = []                    # gi -> (mg, gstart, nb)
    grp_of_block = {}
    mg_groups = [[] for _ in range(MG)]
    for gg in range(MG):
        b0 = int(bbase[gg, 0])
        bn = int(B[gg].sum())
        gs = b0
        while gs < b0 + bn:
            nb = min(GB, b0 + bn - gs)
            gi = len(groups)
            groups.append((gg, gs, nb))
            mg_groups[gg].append(gi)
            for b in range(gs, gs + nb):
                grp_of_block[b] = gi
            gs += nb

    # batches of NQ groups (one per queue)
    def make_batches(gis):
        return [gis[i:i + NQ] for i in range(0, len(gis), NQ)]

    mg_batches = [make_batches(mg_groups[gg]) for gg in range(MG)]

    with tile.TileContext(nc) as tc:
        with tc.tile_pool(name="const", bufs=1) as constp, \
             tc.tile_pool(name="xk", bufs=2) as xkp, \
             tc.tile_pool(name="hc", bufs=3) as hcp, \
             tc.tile_pool(name="gath", bufs=8) as gp, \
             tc.tile_pool(name="ohp", bufs=6) as ohp, \
             tc.tile_pool(name="zt", bufs=2) as zp, \
             tc.tile_pool(name="res", bufs=2) as resp, \
             tc.tile_pool(name="gps", bufs=3, space="PSUM") as gpsp, \
             tc.tile_pool(name="aps", bufs=3, space="PSUM") as apsp, \
             tc.tile_pool(name="eps", bufs=2, space="PSUM") as epsp, \
             tc.tile_pool(name="dram", bufs=1, space="DRAM") as dramp:

            # ---- resident constants ----
            w_sb = constp.tile([128, KT, D_OUT], BF16)
            for kt in range(KT):
                nc.sync.dma_start(out=w_sb[:, kt, :],
                                  in_=w[kt * 128:(kt + 1) * 128, :])
            dis_sb = constp.tile([128, T], F32)
            nc.scalar.dma_start(out=dis_sb[:], in_=dis[:])
            bias_sb = constp.tile([128, D_OUT], F32)
            nc.scalar.dma_start(out=bias_sb[:], in_=biasf[:])
            idx_sb = constp.tile([128, idx_cols], I16)
            nc.scalar.dma_start(out=idx_sb[:], in_=idx[:])

            partial = constp.tile([128, T, D_OUT], BF16)
            sums = constp.tile([128, T], F32)
            lse = constp.tile([128, T], F32)
            nlse = constp.tile([128, T], F32)

            h_loc = [dramp.tile([MG_SZ[gg], D_OUT], BF16, name=f"h_loc{gg}")
                     for gg in range(MG)]
            h_tab = [dramp.tile([MG_ROWS[gg], D_OUT], BF16,
                                addr_space="Shared", name=f"h_tab{gg}")
                     for gg in range(MG)]

            SPQ = NSEM // NQ
            sems = [[nc.alloc_semaphore(f"gsem{q}_{i}") for i in range(SPQ)]
                    for q in range(NQ)]
            sem_uses = [[0] * SPQ for _ in range(NQ)]
            sem_rr = [0] * NQ
            grp_wait = {}

            # ---- phase 1: GEMM ----
            for s in range(NSLAB):
                t0 = s * SLAB
                nt = min(SLAB, T - t0)
                xk = xkp.tile([128, KT, SLABW], BF16, name="xk")
                nc.sync.dma_start(
                    out=xk[:, :, :],
                    in_=xT[:, s * KT * SLABW:(s + 1) * KT * SLABW])
                for ti in range(nt):
                    t_ = t0 + ti
                    ph = gpsp.tile([128, D_OUT], F32, name="ph")
                    for kt in range(KT):
                        nc.tensor.matmul(
                            ph[:], xk[:, kt, ti * 128:(ti + 1) * 128],
                            w_sb[:, kt, :], start=(kt == 0), stop=(kt == KT - 1))
                    hcv = hcp.tile([128, D_OUT], BF16, name="hcv")
                    nc.vector.tensor_copy(hcv[:], ph[:])
                    # self-loop message: h' already carries one dis factor,
                    # the epilogue's z = dis*partial supplies the second.
                    nc.vector.tensor_copy(partial[:, t_, :], ph[:])
                    kk = next(i for i in range(MG)
                              if MG_OFF[i] <= t_ * 128 < MG_OFF[i] + MG_SZ[i])
                    r0 = t_ * 128 - MG_OFF[kk]
                    nc.sync.dma_start(out=h_loc[kk][r0:r0 + 128, :],
                                      in_=hcv[:])

            # ---- phase 2: Pool program ----
            rg = [list(range(C))]

            def emit_ag(kk):
                nc.gpsimd.collective_compute(
                    "AllGather", mybir.AluOpType.bypass,
                    replica_groups=rg,
                    ins=[h_loc[kk].opt()], outs=[h_tab[kk].opt()])

            gtiles = {}

            def emit_prep_batch(gis):
                for j, gi in enumerate(gis):
                    gg, gs, nb = groups[gi]
                    gt = gp.tile([128, GB, D_OUT], BF16, name="gt", tag="gt")
                    gtiles[gi] = gt
                    q = j % NQ
                    si = sem_rr[q] % SPQ
                    sem_rr[q] += 1
                    sem_uses[q][si] += 1
                    grp_wait[gi] = (sems[q][si], 16 * sem_uses[q][si])
                    nc.gpsimd.dma_gather(
                        out_ap=gt[:, :nb, :], in_ap=h_tab[gg][:],
                        idxs_ap=idx_sb[:, 8 * gs:8 * (gs + nb)],
                        num_idxs=nb * 128, num_idxs_reg=nb * 128,
                        elem_size=D_OUT, prepare_only=True, sem=sems[q][si],
                        single_packet=False, queue_num=q)

            def emit_trigger_batch(gis):
                for j in range(min(len(gis), NQ)):
                    nc.gpsimd.trigger_dma(count=None, queue_num=j)

            # strict per-queue prep/trigger alternation; AG triggers woven in
            emit_ag(0)
            emit_prep_batch(mg_batches[0][0])
            emit_ag(1)
            emit_ag(2)
            emit_trigger_batch(mg_batches[0][0])
            for pb in mg_batches[0][1:]:
                emit_prep_batch(pb)
                emit_trigger_batch(pb)
            for gg in (1, 2):
                for pb in mg_batches[gg]:
                    emit_prep_batch(pb)
                    emit_trigger_batch(pb)

            # ---- phase 3: aggregation + epilogue sweep 1 ----
            _oh_tiles = {}
            waited = set()
            for gg in range(MG):
                for tt in range(T):
                    nb_t = int(B[gg, tt])
                    if nb_t == 0:
                        continue
                    ps = apsp.tile([128, D_OUT], F32, name="ps")
                    base = int(bbase[gg, tt])
                    for b in range(nb_t):
                        bi_ = base + b
                        gi = grp_of_block[bi_]
                        if gi not in waited:
                            sem, val = grp_wait[gi]
                            nc.tensor.wait_ge(sem, val)
                            waited.add(gi)
                        _, gs, nb = groups[gi]
                        slot = bi_ - gs
                        oht = _oh_tiles.get(gi)
                        if oht is None:
                            oht = ohp.tile([128, GB * 128], FP8, name="oht")
                            nc.scalar.dma_start(
                                out=oht[:, :nb * 128],
                                in_=oh[:, gs * 128:(gs + nb) * 128])
                            _oh_tiles[gi] = oht
                        nc.tensor.matmul(
                            ps[:], oht[:, slot * 128:(slot + 1) * 128],
                            gtiles[gi][:, slot, :],
                            start=(b == 0), stop=(b == nb_t - 1))
                    nc.vector.tensor_tensor(
                        partial[:, tt, :], partial[:, tt, :], ps[:],
                        mybir.AluOpType.add)
                    if gg == lastg[tt]:
                        zt = zp.tile([128, D_OUT], F32, name="zt")
                        nc.vector.tensor_scalar(
                            zt[:], partial[:, tt, :], dis_sb[:, tt:tt + 1],
                            None, mybir.AluOpType.mult)
                        if has_bias:
                            nc.vector.tensor_tensor(
                                zt[:], zt[:], bias_sb[:], mybir.AluOpType.add)
                        ep = epsp.tile([128, D_OUT], F32, name="ep")
                        nc.scalar.activation(
                            ep[:], zt[:], mybir.ActivationFunctionType.Exp,
                            accum_out=sums[:, tt:tt + 1])

            # ---- phase 4: lse + final sweep ----
            nc.scalar.activation(lse[:], sums[:],
                                 mybir.ActivationFunctionType.Ln)
            nc.vector.tensor_scalar(nlse[:], lse[:], -1.0, None,
                                    mybir.AluOpType.mult)
            for tt in range(T):
                res = resp.tile([128, D_OUT], BF16, name="res")
                if has_bias:
                    zt = zp.tile([128, D_OUT], F32, name="zt2")
                    nc.vector.tensor_scalar(
                        zt[:], partial[:, tt, :], dis_sb[:, tt:tt + 1],
                        nlse[:, tt:tt + 1], mybir.AluOpType.mult,
                        mybir.AluOpType.add)
                    nc.vector.tensor_tensor(res[:], zt[:], bias_sb[:],
                                            mybir.AluOpType.add)
                else:
                    nc.vector.tensor_scalar(
                        res[:], partial[:, tt, :], dis_sb[:, tt:tt + 1],
                        nlse[:, tt:tt + 1], mybir.AluOpType.mult,
                        mybir.AluOpType.add)
                nc.scalar.dma_start(out=out[tt * 128:(tt + 1) * 128, :],
                                    in_=res[:])

    nc.compile()
    return nc


def kernel(x, edge_index, weight, bias):
    global LAST_RESULTS
    x = np.asarray(x, dtype=np.float32)
    weight = np.asarray(weight, dtype=np.float32)
    bias = np.asarray(bias, dtype=np.float32)

    pp = _preprocess(x, edge_index, weight, bias)
    nc = _build(pp["B"], pp["firstg"], pp["lastg"], pp["bbase"],
                pp["has_bias"])

    in_maps = []
    for c in range(C):
        in_maps.append({
            "xT": np.ascontiguousarray(pp["xtile"][c]),
            "w": pp["w"],
            "dis": np.ascontiguousarray(pp["dis"][c]),
            "biasf": pp["bias"],
            "idx": np.ascontiguousarray(pp["idx"][c]),
            "oh": np.ascontiguousarray(pp["oh"][c]),
        })

    res = run_bass_kernel_spmd(nc, in_maps, core_ids=list(range(C)))
    LAST_RESULTS = res

    out = np.empty((N_NODES, D_OUT), np.float32)
    for c in range(C):
        out[c * NLOC:(c + 1) * NLOC] = \
            res.results[c]["out"][:NLOC].astype(np.float32)
    return out
